# revision 17
# baseline (speedup 1.0000x reference)
"""CrossAttention (B=1, S=4096, H=8, DH=40) on 8 Trainium2 NeuronCores.

Sharding: tensor-parallel over the 8 heads — core h computes head h's full
attention plus its partial output projection; the host sums the 8 partials
and adds the bias.

ACT-dense pipeline: the scalar engine's exp over the S*S score matrix
(16.8M elements/core, ~1.15us per [128,1024] ACTIVATE) is the hard floor,
so all PE/DVE work is interleaved per exp-group to keep ACT fed and the
PE warm (no HAM re-throttle gaps).

Per-core dataflow (attention matmuls in bf16; fp32 accumulation in PSUM;
softmax renormalization cancels most of the bf16 rounding of P):
  qkA/qkB [104, 4096] packed projections (q@0|k@64 and k@0|q@64) so the
    two QK^T matmuls of a j-pair land in disjoint PE row groups and run
    concurrently.
  v'      [128s, 41] tiles projected directly in [s, d] orientation
    (stationary xT, moving Wv) — no PE transposes; col 40 = 1.0
    (row-sum trick).
  ST      [128j, 2x512i] = k_j @ q_i.T  (PE, K=40, row-group pair)
  PT      = exp(ST / sqrt(40))          (ScalarE, PSUM->SBUF, bf16)
  O'      accumulate v'_j.T @ PT_j over j: even j at PE cols 0-40 ->
    PSUM parts 0-40, odd j at cols 64-104 -> parts 64-104 (col tiling,
    one PSUM bank; the even/odd merge is folded into the out-proj).
  Y_s     [128, 321] = oU_s.T @ woT2 (two K=41 row-group matmuls
    accumulating; woT2 duplicates Wo rows at 0-39/64-103 and carries a
    ones column so Y[:,320] = the softmax denominator r, transposed for
    free). DVE: rec = 1/Y[:,320], out = Y[:,0:320] * rec -> DMA.
"""

import os

import ml_dtypes
import numpy as np

import concourse.bass as bass
import concourse.mybir as mybir
from concourse import bass_utils

S = 4096
D = 320
H = 8
DH = 40
N_CORES = 8
CHUNK = 512               # i-chunk width (one fp32 PSUM bank)
VW = 65                   # v' stationary width: 40 v cols, ones col 40,
                          # zeros 41-64 (pad M to 65 so every steady-state
                          # matmul shares the (64,128) PE tiling mode)
GJ = 2                    # j-tiles per exp group (2 PSUM banks)
SCALE = float(DH) ** -0.5

F32 = mybir.dt.float32
BF16 = mybir.dt.bfloat16
EXP = mybir.ActivationFunctionType.Exp

_COMPILED = {}

KCH = (128, 128, 64)
QKW = 104
WCOLS = 6 * QKW + 3 * DH + (D + 1)


def _pack_wall(Wq, Wk, Wv, Wo, sl):
    """[wA | wB | wv3 | woT2] packed weights, see _build."""
    bf = ml_dtypes.bfloat16
    wall = np.zeros((128, WCOLS), dtype=np.float32)
    wqT = Wq[sl, :].T
    wkT = Wk[sl, :].T
    wvT = Wv[sl, :].T
    for c, kk in enumerate(KCH):
        o = sum(KCH[:c])
        wall[0:kk, c * QKW:c * QKW + DH] = wqT[o:o + kk]
        wall[0:kk, c * QKW + 64:c * QKW + QKW] = wkT[o:o + kk]
        wall[0:kk, 3 * QKW + c * QKW:3 * QKW + c * QKW + DH] = wkT[o:o + kk]
        wall[0:kk, 3 * QKW + c * QKW + 64:3 * QKW + (c + 1) * QKW] = \
            wqT[o:o + kk]
        wall[0:kk, 6 * QKW + c * DH:6 * QKW + (c + 1) * DH] = wvT[o:o + kk]
    wo0 = 6 * QKW + 3 * DH
    wall[0:DH, wo0:wo0 + D] = Wo[:, sl].T
    wall[DH, wo0 + D] = 1.0
    return wall.astype(bf)


def _split_sync_waits(nc, max_waits=1):
    """This walrus build rejects instructions with more than one sync wait.
    Spill the excess onto same-engine nops placed just before the
    instruction (engine streams execute in program order, so all waits are
    satisfied before the instruction issues)."""
    for f in nc.m.functions:
        for bb in f.blocks:
            out = []
            changed = False
            for inst in bb.instructions:
                si = inst.sync_info
                if si is not None and si.on_wait and len(si.on_wait) > max_waits:
                    waits = list(si.on_wait)
                    for i in range(max_waits, len(waits), max_waits):
                        nop = mybir.InstNoOp(
                            name=nc.get_next_instruction_name(),
                            engine=inst.engine,
                            bass_nofuse=True,
                            sync_info=mybir.SyncInfo(
                                on_wait=waits[i:i + max_waits], on_update=[]),
                        )
                        out.append(nop)
                    inst.sync_info = mybir.SyncInfo(
                        on_wait=waits[:max_waits],
                        on_update=list(si.on_update or []))
                    changed = True
                out.append(inst)
            if changed:
                bb.instructions = out


def _build(s=None, split=True):
    from concourse.tile import TileContext

    s = s or S
    n_chunks = s // CHUNK
    jt = s // 128             # 32 j-tiles
    ng = jt // GJ             # 16 exp groups per i-chunk
    nc = bass.Bass('TRN2', target_bir_lowering=False, debug=False)

    xT_d = nc.dram_tensor('xT', [D, s], BF16, kind='ExternalInput').ap()
    # all weights host-packed into one tensor: [wA | wB | wv3 | woT2]
    KCH = (128, 128, 64)  # K chunks of D=320
    QKW = 104             # packed projection width: q/k @0 | pad | k/q @64
    WCOLS = 3 * QKW + 3 * QKW + 3 * DH + (D + 1)
    wall_d = nc.dram_tensor('wall', [128, WCOLS], BF16,
                            kind='ExternalInput').ap()
    out_d = nc.dram_tensor('out', [s, D], F32, kind='ExternalOutput').ap()

    with TileContext(nc) as tc:
        with tc.tile_pool(name='const', bufs=1) as cpool, \
             tc.tile_pool(name='big', bufs=1) as big, \
             tc.tile_pool(name='pt', bufs=40) as ptp, \
             tc.tile_pool(name='ou', bufs=2) as oup, \
             tc.tile_pool(name='work', bufs=3) as wkp, \
             tc.tile_pool(name='ps_st', bufs=2, space='PSUM') as ps_st, \
             tc.tile_pool(name='ps_small', bufs=2, space='PSUM') as ps_small, \
             tc.tile_pool(name='ps_av', bufs=1, space='PSUM') as ps_av:

            # ---- constants & inputs ----
            # All weights in ONE DMA (descriptor issue costs ~600ns each
            # on the sync queue; many small DMAs serialized the prologue):
            # wA = [wq | 0 | wk], wB = [wk | 0 | wq] per K-chunk (packed
            # projections giving qkA = [q@0-39 | k@64-103], qkB =
            # [k@0-39 | q@64-103]), wv per K-chunk, woT2 = Wo^T plus a
            # ones column (lands the softmax denominator r at yp[:, 320],
            # transposed for free by the out-proj matmul). Host-prepared.
            wall = cpool.tile([128, WCOLS], BF16, tag='wall')
            nc.sync.dma_start(wall[:, :], wall_d)
            wA = wall[:, 0:3 * QKW]
            wB = wall[:, 3 * QKW:6 * QKW]
            wv_sb = wall[:, 6 * QKW:6 * QKW + 3 * DH]
            woT2 = wall[0:DH + 1, 6 * QKW + 3 * DH:WCOLS]

            xt0 = big.tile([128, s], BF16, tag='xt0')
            xt1 = big.tile([128, s], BF16, tag='xt1')
            xt2 = big.tile([64, s], BF16, tag='xt2')
            xts = (xt0, xt1, xt2)
            # chunks 0-2 separately (the prologue projections need them
            # early), issued from three engine queues in parallel
            # (descriptor issue costs ~600ns per dma_start on a queue);
            # then the rest in 3 large DMAs
            for c in range(3):
                cs = slice(c * CHUNK, (c + 1) * CHUNK)
                nc.sync.dma_start(xt0[:, cs], xT_d[0:128, cs])
                nc.gpsimd.dma_start(xt1[:, cs], xT_d[128:256, cs])
                nc.scalar.dma_start(xt2[:, cs], xT_d[256:320, cs])
            nc.sync.dma_start(xt0[:, 3 * CHUNK:s], xT_d[0:128, 3 * CHUNK:s])
            nc.gpsimd.dma_start(xt1[:, 3 * CHUNK:s],
                                xT_d[128:256, 3 * CHUNK:s])
            nc.scalar.dma_start(xt2[:, 3 * CHUNK:s],
                                xT_d[256:320, 3 * CHUNK:s])

            qkA = big.tile([QKW, s], BF16, tag='qkA')
            qkB = big.tile([QKW, s], BF16, tag='qkB')
            # v' tiles [128 s, 65]: cols 0-39 = v (projected in [s, d]
            # orientation), col 40 = 1.0 (row-sum column), 41-64 zero.
            vsb = big.tile([128, jt, VW], BF16, tag='vsb')
            nc.vector.memset(vsb[:, :, DH:DH + 1], 1.0)
            nc.vector.memset(vsb[:, :, DH + 1:VW], 0.0)

            # ---- helpers ----
            def proj_qk(dst, w_sb, c):
                ps = ps_small.tile([QKW, CHUNK], F32, tag='small')
                for ci, kk in enumerate(KCH):
                    nc.tensor.matmul(
                        ps[:, :],
                        w_sb[0:kk, ci * QKW:(ci + 1) * QKW],
                        xts[ci][0:kk, c * CHUNK:(c + 1) * CHUNK],
                        start=(ci == 0), stop=(ci == 2))
                nc.vector.tensor_copy(dst[:, c * CHUNK:(c + 1) * CHUNK],
                                      ps[:, :])

            def proj_v(t):
                # v'_t [128 s, 40] = x[s-tile t] @ Wv^T: stationary xT
                # K-chunks, moving Wv.
                ps = ps_small.tile([128, DH], F32, tag='small')
                for ci, kk in enumerate(KCH):
                    nc.tensor.matmul(
                        ps[:, :],
                        xts[ci][0:kk, t * 128:(t + 1) * 128],
                        wv_sb[0:kk, ci * DH:(ci + 1) * DH],
                        start=(ci == 0), stop=(ci == 2))
                nc.vector.tensor_copy(vsb[:, t:t + 1, 0:DH], ps[:, :])

            st_tiles = {}

            def st_group(c, g, base0=False):
                # scores for j-pair (2g, 2g+1) over i-chunk c.
                st = ps_st.tile([128, GJ * CHUNK], F32, tag='st2')
                cs = slice(c * CHUNK, (c + 1) * CHUNK)
                j0, j1 = GJ * g, GJ * g + 1
                nc.tensor.matmul(
                    st[:, 0:CHUNK],
                    qkB[0:DH, j0 * 128:(j0 + 1) * 128], qkA[0:DH, cs],
                    start=True, stop=True)
                if base0:
                    # qkA's k@64 rows are not projected yet: base-0 form
                    # (serializes the pair; earliest chunk-0 groups only).
                    nc.tensor.matmul(
                        st[:, CHUNK:2 * CHUNK],
                        qkB[0:DH, j1 * 128:(j1 + 1) * 128], qkA[0:DH, cs],
                        start=True, stop=True)
                else:
                    nc.tensor.matmul(
                        st[:, CHUNK:2 * CHUNK],
                        qkA[64:64 + DH, j1 * 128:(j1 + 1) * 128],
                        qkB[64:64 + DH, cs],
                        start=True, stop=True)
                st_tiles[(c, g)] = st

            pt_tiles = {}

            def exp_group(c, g):
                st = st_tiles.pop((c, g))
                pt = ptp.tile([128, GJ * CHUNK], BF16, tag='pt')
                nc.scalar.activation(pt[:, :], st[:, :], EXP, scale=SCALE)
                pt_tiles[(c, g)] = pt

            def av_group(c, g, avA, avB):
                # Row-split K=64 halves at PE row groups 0/64 (concurrent,
                # same (64,128) mode as ST/out-proj); two partial
                # accumulators in separate banks, merged on DVE at chunk
                # end.
                pt = pt_tiles.pop((c, g))
                for jj in range(GJ):
                    j = GJ * g + jj
                    pcs = slice(jj * CHUNK, (jj + 1) * CHUNK)
                    nc.tensor.matmul(
                        avA[:, :], vsb[0:64, j:j + 1, :], pt[0:64, pcs],
                        start=(g == 0 and jj == 0),
                        stop=(g == ng - 1 and jj == GJ - 1),
                        tile_position=(0, 0))
                    nc.tensor.matmul(
                        avB[:, :], vsb[64:128, j:j + 1, :], pt[64:128, pcs],
                        start=(g == 0 and jj == 0),
                        stop=(g == ng - 1 and jj == GJ - 1),
                        tile_position=(64, 0))

            def merge_chunk(avA, avB):
                # Merge the two K-half partials (rows 0-40 carry data; the
                # ones row 40 holds the softmax denominator r). Emitted in
                # the same slot as the chunk's last AV so the avA/avB
                # banks hand off cleanly to the next chunk.
                m1 = wkp.tile([DH + 1, CHUNK], F32, tag='m1')
                nc.vector.tensor_copy(m1[:, :], avA[0:DH + 1, :])
                oU = oup.tile([DH + 1, CHUNK], BF16, tag='oU')
                nc.vector.tensor_tensor(
                    out=oU[:, :], in0=avB[0:DH + 1, :], in1=m1[:, :],
                    op=mybir.AluOpType.add)
                return oU

            def out_piece(c, s2, oU):
                # One s-tile of the output projection: yp[:, 320] = r
                # transposed for free via woT2's ones column; normalize
                # with a per-partition reciprocal folded into the
                # PSUM->SBUF copy. One piece per slot.
                st_i = c * (CHUNK // 128) + s2
                sl = slice(s2 * 128, (s2 + 1) * 128)
                yp = ps_small.tile([128, D + 1], F32, tag='small')
                nc.tensor.matmul(yp[:, :], oU[:, sl], woT2[:, :],
                                 start=True, stop=True)
                rec = wkp.tile([128, 1], F32, tag='rec')
                nc.vector.reciprocal(rec[:, :], yp[:, D:D + 1])
                ysb = wkp.tile([128, D], F32, tag='ysb')
                nc.vector.tensor_scalar(
                    out=ysb[:, :], in0=yp[:, 0:D], scalar1=rec[:, :],
                    scalar2=None, op0=mybir.AluOpType.mult)
                nc.sync.dma_start(out_d[st_i * 128:(st_i + 1) * 128, :],
                                  ysb[:, :])

            # ---- prologue: projections interleaved with chunk-0 scores
            # so the first exp issues ~2.5us in and ACT stays fed. The
            # first two groups use the base-0 ST form (qkA's k@64 rows are
            # not projected yet); v' tiles 0-3 cover AV groups 0-1.
            proj_qk(qkB, wB, 0)
            proj_qk(qkA, wA, 0)
            st_group(0, 0, base0=True)
            proj_qk(qkB, wB, 1)
            st_group(0, 1, base0=True)
            proj_qk(qkB, wB, 2)
            exp_group(0, 0)
            exp_group(0, 1)
            for t in range(4):
                proj_v(t)

            # chunk-0 filler schedule: one qkA/qkB projection per slot
            # (slots 0-11), ordered so ST(0, g+2)'s k@64 stationary (qkA
            # chunk (2g+5)//4) and k@0 stationary (qkB chunk (g+2)//2)
            # are always emitted before the ST that reads them; one v'
            # tile per slot (slots 0-27; AV waits until slot 28).
            qk_fill = [('A', 1), ('B', 3), ('A', 2), ('B', 4), ('A', 3),
                       ('B', 5), ('A', 4), ('B', 6), ('A', 5), ('B', 7),
                       ('A', 6), ('A', 7)]

            # ---- main loop: flat software pipeline over the 128
            # (chunk, group) stages. ST/exp run 2 groups ahead of the ACT
            # stream; AV processing starts only after every (128,128)-mode
            # projection is emitted (slot 28) -- PE tiling-mode switches
            # inside an open AV accumulation group crash the device -- and
            # then catches back up to lag ~4 by occasionally running two
            # groups per slot (the PE has ~300ns/slot of slack).
            NG = n_chunks * ng
            AV_START = 34
            av_ptr = 0
            avA = avB = None
            oc_items = []
            slot = 0
            while av_ptr < NG or oc_items:
                if slot < len(qk_fill):
                    kind, a = qk_fill[slot]
                    proj_qk(qkA if kind == 'A' else qkB,
                            wA if kind == 'A' else wB, a)
                # v' projections in slots 12-33 (after the qk fillers, at
                # most ~1.4/slot) so the early slots stay under the ACT
                # budget; AV consumption starts at slot 34.
                if 12 <= slot < 34:
                    t = 4 + ((slot - 12) * 28) // 22
                    t2 = 4 + ((slot - 11) * 28) // 22
                    for tt in range(t, min(t2, jt)):
                        proj_v(tt)
                gs = slot + 2
                if gs < NG:
                    st_group(gs // ng, gs % ng)
                    exp_group(gs // ng, gs % ng)
                navs = 0
                if slot >= AV_START:
                    navs = 1
                    if gs >= NG:
                        navs = 4
                    elif gs - av_ptr > 4 and slot % 2 == 1:
                        navs = 2
                for _ in range(navs):
                    if av_ptr >= NG:
                        break
                    c, g = av_ptr // ng, av_ptr % ng
                    if g == 0:
                        avA = ps_av.tile([VW, CHUNK], F32, tag='avA',
                                         name='avA')
                        avB = ps_av.tile([VW, CHUNK], F32, tag='avB',
                                         name='avB')
                    av_group(c, g, avA, avB)
                    if g == ng - 1:
                        oU = merge_chunk(avA, avB)
                        for s2 in range(CHUNK // 128):
                            oc_items.append((c, s2, oU))
                    av_ptr += 1
                for _ in range(2 if gs >= NG else 1):
                    if oc_items:
                        out_piece(*oc_items.pop(0))
                slot += 1

    if split:
        _split_sync_waits(nc)
    return nc


def kernel(x, Wq, Wk, Wv, Wo, bo):
    x = np.asarray(x, dtype=np.float32)
    Wq = np.asarray(Wq, dtype=np.float32)
    Wk = np.asarray(Wk, dtype=np.float32)
    Wv = np.asarray(Wv, dtype=np.float32)
    Wo = np.asarray(Wo, dtype=np.float32)
    bo = np.asarray(bo, dtype=np.float32)

    if 'nc' not in _COMPILED:
        _COMPILED['nc'] = _build()
    nc = _COMPILED['nc']

    bf = ml_dtypes.bfloat16
    xT = np.ascontiguousarray(x.reshape(S, D).T).astype(bf)
    in_maps = []
    for h in range(N_CORES):
        sl = slice(h * DH, (h + 1) * DH)
        in_maps.append({'xT': xT, 'wall': _pack_wall(Wq, Wk, Wv, Wo, sl)})

    trace = bool(os.environ.get('BASS_KERNEL_TRACE'))

    def _run():
        return bass_utils.run_bass_kernel_spmd(
            nc, in_maps, core_ids=list(range(N_CORES)), trace=trace,
            tmpdir=os.environ.get('BASS_KERNEL_TRACE_DIR') or None)

    try:
        res = _run()
    except Exception:
        # A previously crashed NEFF can leave the device unrecoverable; the
        # failed attempt clears it, so one retry is usually enough.
        res = _run()
    _COMPILED['last_res'] = res

    acc = res.results[0]['out'].astype(np.float32).copy()
    for h in range(1, N_CORES):
        acc += res.results[h]['out']
    acc += bo[None, :]
    return acc.reshape(1, S, D)


# revision 19
# speedup vs baseline: 1.0079x; 1.0079x over previous
"""CrossAttention (B=1, S=4096, H=8, DH=40) on 8 Trainium2 NeuronCores.

Sharding: tensor-parallel over the 8 heads — core h computes head h's full
attention plus its partial output projection; the host sums the 8 partials
and adds the bias.

ACT-dense pipeline: the scalar engine's exp over the S*S score matrix
(16.8M elements/core, ~1.15us per [128,1024] ACTIVATE) is the hard floor,
so all PE/DVE work is interleaved per exp-group to keep ACT fed and the
PE warm (no HAM re-throttle gaps).

Per-core dataflow (attention matmuls in bf16; fp32 accumulation in PSUM;
softmax renormalization cancels most of the bf16 rounding of P):
  qkA/qkB [104, 4096] packed projections (q@0|k@64 and k@0|q@64) so the
    two QK^T matmuls of a j-pair land in disjoint PE row groups and run
    concurrently.
  v'      [128s, 41] tiles projected directly in [s, d] orientation
    (stationary xT, moving Wv) — no PE transposes; col 40 = 1.0
    (row-sum trick).
  ST      [128j, 2x512i] = k_j @ q_i.T  (PE, K=40, row-group pair)
  PT      = exp(ST / sqrt(40))          (ScalarE, PSUM->SBUF, bf16)
  O'      accumulate v'_j.T @ PT_j over j: even j at PE cols 0-40 ->
    PSUM parts 0-40, odd j at cols 64-104 -> parts 64-104 (col tiling,
    one PSUM bank; the even/odd merge is folded into the out-proj).
  Y_s     [128, 321] = oU_s.T @ woT2 (two K=41 row-group matmuls
    accumulating; woT2 duplicates Wo rows at 0-39/64-103 and carries a
    ones column so Y[:,320] = the softmax denominator r, transposed for
    free). DVE: rec = 1/Y[:,320], out = Y[:,0:320] * rec -> DMA.
"""

import os

import ml_dtypes
import numpy as np

import concourse.bass as bass
import concourse.mybir as mybir
from concourse import bass_utils

S = 4096
D = 320
H = 8
DH = 40
N_CORES = 8
CHUNK = 512               # i-chunk width (one fp32 PSUM bank)
VW = 65                   # v' stationary width: 40 v cols, ones col 40,
                          # zeros 41-64 (pad M to 65 so every steady-state
                          # matmul shares the (64,128) PE tiling mode)
GJ = 2                    # j-tiles per exp group (2 PSUM banks)
SCALE = float(DH) ** -0.5

F32 = mybir.dt.float32
BF16 = mybir.dt.bfloat16
EXP = mybir.ActivationFunctionType.Exp

_COMPILED = {}

KCH = (128, 128, 64)
QKW = 104
WCOLS = 6 * QKW + 3 * DH + (D + 1)


def _pack_wall(Wq, Wk, Wv, Wo, sl):
    """[wA | wB | wv3 | woT2] packed weights, see _build."""
    bf = ml_dtypes.bfloat16
    wall = np.zeros((128, WCOLS), dtype=np.float32)
    wqT = Wq[sl, :].T
    wkT = Wk[sl, :].T
    wvT = Wv[sl, :].T
    for c, kk in enumerate(KCH):
        o = sum(KCH[:c])
        wall[0:kk, c * QKW:c * QKW + DH] = wqT[o:o + kk]
        wall[0:kk, c * QKW + 64:c * QKW + QKW] = wkT[o:o + kk]
        wall[0:kk, 3 * QKW + c * QKW:3 * QKW + c * QKW + DH] = wkT[o:o + kk]
        wall[0:kk, 3 * QKW + c * QKW + 64:3 * QKW + (c + 1) * QKW] = \
            wqT[o:o + kk]
        wall[0:kk, 6 * QKW + c * DH:6 * QKW + (c + 1) * DH] = wvT[o:o + kk]
    wo0 = 6 * QKW + 3 * DH
    wall[0:DH, wo0:wo0 + D] = Wo[:, sl].T
    wall[DH, wo0 + D] = 1.0
    return wall.astype(bf)


def _split_sync_waits(nc, max_waits=1):
    """This walrus build rejects instructions with more than one sync wait.
    Spill the excess onto same-engine nops placed just before the
    instruction (engine streams execute in program order, so all waits are
    satisfied before the instruction issues)."""
    for f in nc.m.functions:
        for bb in f.blocks:
            out = []
            changed = False
            for inst in bb.instructions:
                si = inst.sync_info
                if si is not None and si.on_wait and len(si.on_wait) > max_waits:
                    waits = list(si.on_wait)
                    for i in range(max_waits, len(waits), max_waits):
                        nop = mybir.InstNoOp(
                            name=nc.get_next_instruction_name(),
                            engine=inst.engine,
                            bass_nofuse=True,
                            sync_info=mybir.SyncInfo(
                                on_wait=waits[i:i + max_waits], on_update=[]),
                        )
                        out.append(nop)
                    inst.sync_info = mybir.SyncInfo(
                        on_wait=waits[:max_waits],
                        on_update=list(si.on_update or []))
                    changed = True
                out.append(inst)
            if changed:
                bb.instructions = out


def _build(s=None, split=True):
    from concourse.tile import TileContext

    s = s or S
    n_chunks = s // CHUNK
    jt = s // 128             # 32 j-tiles
    ng = jt // GJ             # 16 exp groups per i-chunk
    nc = bass.Bass('TRN2', target_bir_lowering=False, debug=False)

    xT_d = nc.dram_tensor('xT', [D, s], BF16, kind='ExternalInput').ap()
    # all weights host-packed into one tensor: [wA | wB | wv3 | woT2]
    KCH = (128, 128, 64)  # K chunks of D=320
    QKW = 104             # packed projection width: q/k @0 | pad | k/q @64
    WCOLS = 3 * QKW + 3 * QKW + 3 * DH + (D + 1)
    wall_d = nc.dram_tensor('wall', [128, WCOLS], BF16,
                            kind='ExternalInput').ap()
    out_d = nc.dram_tensor('out', [s, D], F32, kind='ExternalOutput').ap()

    with TileContext(nc) as tc:
        with tc.tile_pool(name='const', bufs=1) as cpool, \
             tc.tile_pool(name='big', bufs=1) as big, \
             tc.tile_pool(name='pt', bufs=34) as ptp, \
             tc.tile_pool(name='ou', bufs=2) as oup, \
             tc.tile_pool(name='work', bufs=3) as wkp, \
             tc.tile_pool(name='ps_st', bufs=2, space='PSUM') as ps_st, \
             tc.tile_pool(name='ps_small', bufs=2, space='PSUM') as ps_small, \
             tc.tile_pool(name='ps_av', bufs=1, space='PSUM') as ps_av:

            # ---- constants & inputs ----
            # All weights in ONE DMA (descriptor issue costs ~600ns each
            # on the sync queue; many small DMAs serialized the prologue):
            # wA = [wq | 0 | wk], wB = [wk | 0 | wq] per K-chunk (packed
            # projections giving qkA = [q@0-39 | k@64-103], qkB =
            # [k@0-39 | q@64-103]), wv per K-chunk, woT2 = Wo^T plus a
            # ones column (lands the softmax denominator r at yp[:, 320],
            # transposed for free by the out-proj matmul). Host-prepared.
            wall = cpool.tile([128, WCOLS], BF16, tag='wall')
            nc.sync.dma_start(wall[:, :], wall_d)
            wA = wall[:, 0:3 * QKW]
            wB = wall[:, 3 * QKW:6 * QKW]
            wv_sb = wall[:, 6 * QKW:6 * QKW + 3 * DH]
            woT2 = wall[0:DH + 1, 6 * QKW + 3 * DH:WCOLS]

            xt0 = big.tile([128, s], BF16, tag='xt0')
            xt1 = big.tile([128, s], BF16, tag='xt1')
            xt2 = big.tile([64, s], BF16, tag='xt2')
            xts = (xt0, xt1, xt2)
            # chunks 0-2 separately (the prologue projections need them
            # early), issued from three engine queues in parallel
            # (descriptor issue costs ~600ns per dma_start on a queue);
            # then the rest in 3 large DMAs
            for c in range(3):
                cs = slice(c * CHUNK, (c + 1) * CHUNK)
                nc.sync.dma_start(xt0[:, cs], xT_d[0:128, cs])
                nc.gpsimd.dma_start(xt1[:, cs], xT_d[128:256, cs])
                nc.gpsimd.dma_start(xt2[:, cs], xT_d[256:320, cs])
            nc.sync.dma_start(xt0[:, 3 * CHUNK:s], xT_d[0:128, 3 * CHUNK:s])
            nc.gpsimd.dma_start(xt1[:, 3 * CHUNK:s],
                                xT_d[128:256, 3 * CHUNK:s])
            nc.gpsimd.dma_start(xt2[:, 3 * CHUNK:s],
                                xT_d[256:320, 3 * CHUNK:s])

            qkA = big.tile([QKW, s], BF16, tag='qkA')
            qkB = big.tile([QKW, s], BF16, tag='qkB')
            # v' tiles [128 s, 65]: cols 0-39 = v (projected in [s, d]
            # orientation), col 40 = 1.0 (row-sum column), 41-64 zero.
            vsb = big.tile([128, jt, VW], BF16, tag='vsb')
            nc.vector.memset(vsb[:, :, DH:DH + 1], 1.0)
            nc.vector.memset(vsb[:, :, DH + 1:VW], 0.0)

            # ---- helpers ----
            def proj_qk(dst, w_sb, c):
                ps = ps_small.tile([QKW, CHUNK], F32, tag='small')
                for ci, kk in enumerate(KCH):
                    nc.tensor.matmul(
                        ps[:, :],
                        w_sb[0:kk, ci * QKW:(ci + 1) * QKW],
                        xts[ci][0:kk, c * CHUNK:(c + 1) * CHUNK],
                        start=(ci == 0), stop=(ci == 2))
                nc.vector.tensor_copy(dst[:, c * CHUNK:(c + 1) * CHUNK],
                                      ps[:, :])

            def proj_v4(t0):
                # Four v' tiles [128 s, 40] = x[s-tile] @ Wv^T per psum
                # tile (stationary xT K-chunks, moving Wv; accumulation
                # groups sequential per 40-col range) with ONE strided
                # cast into vsb -- amortizes the proj->cast->proj
                # semaphore hops that made per-tile projection ~1.2us.
                ps = ps_small.tile([128, 4 * DH], F32, tag='small')
                for t in range(t0, t0 + 4):
                    o = (t - t0) * DH
                    for ci, kk in enumerate(KCH):
                        nc.tensor.matmul(
                            ps[:, o:o + DH],
                            xts[ci][0:kk, t * 128:(t + 1) * 128],
                            wv_sb[0:kk, ci * DH:(ci + 1) * DH],
                            start=(ci == 0), stop=(ci == 2))
                nc.vector.tensor_copy(
                    vsb[:, t0:t0 + 4, 0:DH],
                    ps[:, :].rearrange('p (t d) -> p t d', t=4))

            st_tiles = {}

            def st_group(c, g, base0=False):
                # scores for j-pair (2g, 2g+1) over i-chunk c.
                st = ps_st.tile([128, GJ * CHUNK], F32, tag='st2')
                cs = slice(c * CHUNK, (c + 1) * CHUNK)
                j0, j1 = GJ * g, GJ * g + 1
                nc.tensor.matmul(
                    st[:, 0:CHUNK],
                    qkB[0:DH, j0 * 128:(j0 + 1) * 128], qkA[0:DH, cs],
                    start=True, stop=True)
                if base0:
                    # qkA's k@64 rows are not projected yet: base-0 form
                    # (serializes the pair; earliest chunk-0 groups only).
                    nc.tensor.matmul(
                        st[:, CHUNK:2 * CHUNK],
                        qkB[0:DH, j1 * 128:(j1 + 1) * 128], qkA[0:DH, cs],
                        start=True, stop=True)
                else:
                    nc.tensor.matmul(
                        st[:, CHUNK:2 * CHUNK],
                        qkA[64:64 + DH, j1 * 128:(j1 + 1) * 128],
                        qkB[64:64 + DH, cs],
                        start=True, stop=True)
                st_tiles[(c, g)] = st

            pt_tiles = {}

            def exp_group(c, g):
                st = st_tiles.pop((c, g))
                pt = ptp.tile([128, GJ * CHUNK], BF16, tag='pt')
                nc.scalar.activation(pt[:, :], st[:, :], EXP, scale=SCALE)
                pt_tiles[(c, g)] = pt

            def av_group(c, g, avA, avB):
                # Row-split K=64 halves at PE row groups 0/64 (concurrent,
                # same (64,128) mode as ST/out-proj); two partial
                # accumulators in separate banks, merged on DVE at chunk
                # end.
                pt = pt_tiles.pop((c, g))
                for jj in range(GJ):
                    j = GJ * g + jj
                    pcs = slice(jj * CHUNK, (jj + 1) * CHUNK)
                    nc.tensor.matmul(
                        avA[:, :], vsb[0:64, j:j + 1, :], pt[0:64, pcs],
                        start=(g == 0 and jj == 0),
                        stop=(g == ng - 1 and jj == GJ - 1),
                        tile_position=(0, 0))
                    nc.tensor.matmul(
                        avB[:, :], vsb[64:128, j:j + 1, :], pt[64:128, pcs],
                        start=(g == 0 and jj == 0),
                        stop=(g == ng - 1 and jj == GJ - 1),
                        tile_position=(64, 0))

            def merge_chunk(avA, avB):
                # Merge the two K-half partials (rows 0-40 carry data; the
                # ones row 40 holds the softmax denominator r). Emitted in
                # the same slot as the chunk's last AV so the avA/avB
                # banks hand off cleanly to the next chunk.
                m1 = wkp.tile([DH + 1, CHUNK], F32, tag='m1')
                nc.vector.tensor_copy(m1[:, :], avA[0:DH + 1, :])
                oU = oup.tile([DH + 1, CHUNK], BF16, tag='oU')
                nc.vector.tensor_tensor(
                    out=oU[:, :], in0=avB[0:DH + 1, :], in1=m1[:, :],
                    op=mybir.AluOpType.add)
                return oU

            def out_piece(c, s2, oU):
                # One s-tile of the output projection: yp[:, 320] = r
                # transposed for free via woT2's ones column; normalize
                # with a per-partition reciprocal folded into the
                # PSUM->SBUF copy. One piece per slot.
                st_i = c * (CHUNK // 128) + s2
                sl = slice(s2 * 128, (s2 + 1) * 128)
                yp = ps_small.tile([128, D + 1], F32, tag='small')
                nc.tensor.matmul(yp[:, :], oU[:, sl], woT2[:, :],
                                 start=True, stop=True)
                rec = wkp.tile([128, 1], F32, tag='rec')
                nc.vector.reciprocal(rec[:, :], yp[:, D:D + 1])
                ysb = wkp.tile([128, D], F32, tag='ysb')
                nc.vector.tensor_scalar(
                    out=ysb[:, :], in0=yp[:, 0:D], scalar1=rec[:, :],
                    scalar2=None, op0=mybir.AluOpType.mult)
                nc.sync.dma_start(out_d[st_i * 128:(st_i + 1) * 128, :],
                                  ysb[:, :])

            # ---- prologue: projections interleaved with chunk-0 scores
            # so the first exp issues ~2.5us in and ACT stays fed. The
            # first two groups use the base-0 ST form (qkA's k@64 rows are
            # not projected yet); v' tiles 0-3 cover AV groups 0-1.
            proj_qk(qkB, wB, 0)
            proj_qk(qkA, wA, 0)
            st_group(0, 0, base0=True)
            proj_qk(qkB, wB, 1)
            st_group(0, 1, base0=True)
            proj_qk(qkB, wB, 2)
            exp_group(0, 0)
            exp_group(0, 1)
            proj_v4(0)

            # chunk-0 filler schedule: one qkA/qkB projection per slot
            # (slots 0-11), ordered so ST(0, g+2)'s k@64 stationary (qkA
            # chunk (2g+5)//4) and k@0 stationary (qkB chunk (g+2)//2)
            # are always emitted before the ST that reads them; one v'
            # tile per slot (slots 0-27; AV waits until slot 28).
            qk_fill = [('A', 1), ('B', 3), ('A', 2), ('B', 4), ('A', 3),
                       ('B', 5), ('A', 4), ('B', 6), ('A', 5), ('B', 7),
                       ('A', 6), ('A', 7)]

            # ---- main loop: flat software pipeline over the 128
            # (chunk, group) stages. ST/exp run 2 groups ahead of the ACT
            # stream; AV processing starts only after every (128,128)-mode
            # projection is emitted (slot 28) -- PE tiling-mode switches
            # inside an open AV accumulation group crash the device -- and
            # then catches back up to lag ~4 by occasionally running two
            # groups per slot (the PE has ~300ns/slot of slack).
            NG = n_chunks * ng
            AV_START = 28
            av_ptr = 0
            avA = avB = None
            oc_items = []
            slot = 0
            while av_ptr < NG or oc_items:
                if slot < len(qk_fill):
                    kind, a = qk_fill[slot]
                    proj_qk(qkA if kind == 'A' else qkB,
                            wA if kind == 'A' else wB, a)
                # one 4-tile v' projection batch every other slot in
                # slots 12-26 (after the qk fillers)
                if 12 <= slot < 26 and slot % 2 == 0:
                    proj_v4(4 + 2 * (slot - 12))
                gs = slot + 2
                if gs < NG:
                    st_group(gs // ng, gs % ng)
                    exp_group(gs // ng, gs % ng)
                navs = 0
                if slot >= AV_START:
                    navs = 1
                    if gs >= NG:
                        navs = 4
                    elif gs - av_ptr > 4 and slot % 2 == 1:
                        navs = 2
                for _ in range(navs):
                    if av_ptr >= NG:
                        break
                    c, g = av_ptr // ng, av_ptr % ng
                    if g == 0:
                        avA = ps_av.tile([VW, CHUNK], F32, tag='avA',
                                         name='avA')
                        avB = ps_av.tile([VW, CHUNK], F32, tag='avB',
                                         name='avB')
                    av_group(c, g, avA, avB)
                    if g == ng - 1:
                        oU = merge_chunk(avA, avB)
                        for s2 in range(CHUNK // 128):
                            oc_items.append((c, s2, oU))
                    av_ptr += 1
                for _ in range(2 if gs >= NG else 1):
                    if oc_items:
                        out_piece(*oc_items.pop(0))
                slot += 1

    if split:
        _split_sync_waits(nc)
    return nc


def kernel(x, Wq, Wk, Wv, Wo, bo):
    x = np.asarray(x, dtype=np.float32)
    Wq = np.asarray(Wq, dtype=np.float32)
    Wk = np.asarray(Wk, dtype=np.float32)
    Wv = np.asarray(Wv, dtype=np.float32)
    Wo = np.asarray(Wo, dtype=np.float32)
    bo = np.asarray(bo, dtype=np.float32)

    if 'nc' not in _COMPILED:
        _COMPILED['nc'] = _build()
    nc = _COMPILED['nc']

    bf = ml_dtypes.bfloat16
    xT = np.ascontiguousarray(x.reshape(S, D).T).astype(bf)
    in_maps = []
    for h in range(N_CORES):
        sl = slice(h * DH, (h + 1) * DH)
        in_maps.append({'xT': xT, 'wall': _pack_wall(Wq, Wk, Wv, Wo, sl)})

    trace = bool(os.environ.get('BASS_KERNEL_TRACE'))

    def _run():
        return bass_utils.run_bass_kernel_spmd(
            nc, in_maps, core_ids=list(range(N_CORES)), trace=trace,
            tmpdir=os.environ.get('BASS_KERNEL_TRACE_DIR') or None)

    try:
        res = _run()
    except Exception:
        # A previously crashed NEFF can leave the device unrecoverable; the
        # failed attempt clears it, so one retry is usually enough.
        res = _run()
    _COMPILED['last_res'] = res

    acc = res.results[0]['out'].astype(np.float32).copy()
    for h in range(1, N_CORES):
        acc += res.results[h]['out']
    acc += bo[None, :]
    return acc.reshape(1, S, D)
